# revision 77
# baseline (speedup 1.0000x reference)
"""Multi-head attention on 8 Trainium2 NeuronCores.

Problem: B=2, S=2048, D=1024, H=16 heads (head_dim 64), boolean mask,
per-head gate, QKV/out linear projections.

Sharding: core c handles batch b=c//4 and heads 4*(c%4)..4*(c%4)+3.
Each core computes its 4 heads' attention and the partial output
projection; the host sums the 4 partials per batch and adds the
constant terms (bo, and the bv/gate contribution).

Device-side design (v2):
  - scores are computed TRANSPOSED [sk, sq]; exp is a pure ACT pass,
    mask is a multiplicative bf16 tensor_mul on DVE.
  - PV matmul is FLIPPED: pm chunks stationary, vh [sk, hd] moving, so
    each accumulation step emits 64 columns instead of 512 (the PE cost
    model charges by moving columns only). Output lands natural
    [sq, hd]; the denominator accumulates from 1-column matmuls against
    a ones vector, so normalization is a per-partition tensor_scalar.
  - normalized [sq, hd] tiles go back to concat^T layout via the XBAR
    transpose DMA.
  - accumulation chains share PSUM banks via memset + start=False
    (correct both in CoreSim's zero-region model and on HW).
  - the whole program is a software-pipelined chunk stream: exp chunk
    i+1's scores are emitted before exp chunk i; Q/K c1 projections,
    the V projection (octants) and the O-projection run as
    cost-budgeted fillers inside the stream so ACT (the 133us exp
    roofline) is fed from ~16us onward.
"""

import sys

if "/opt/trn_rl_repo" not in sys.path:
    sys.path.insert(0, "/opt/trn_rl_repo")

import numpy as np

import concourse.bass as bass
import concourse.bacc as bacc
import concourse.mybir as mybir
import concourse.tile as tile
from concourse.bass_utils import run_bass_kernel_spmd

BF16 = mybir.dt.float16  # fp16: same speed as bf16, 3 more mantissa bits
F32 = mybir.dt.float32
NPBF16 = np.float16

P = 128
B, S, D = 2, 2048, 1024
HEADS, HD = 16, 64
NCORES = 8
NH = HEADS // (NCORES // B)  # heads per core = 4
COLS = NH * HD               # 256 concat columns per core
DK = D // P                  # 8 contraction chunks for the projections
SKT = S // P                 # 16 key chunks
SQB = 1024                   # query block width in the attention loop
NSQB = S // SQB
SQC = SQB // P               # 8 sq chunks of 128 per block

FILL_NS = 450                # filler budget per exp chunk (ns of PE time)
XV1_CHUNK = 16               # earliest chunk for octants on the xv second half
XQ1_CHUNK = 24               # earliest chunk for chains on the xq second half

_CACHE = {}


def _build_program():
    nc = bacc.Bacc("TRN2", debug=False)

    xqT = nc.declare_dram_parameter("xqT", [D, S], BF16, isOutput=False)
    xkT = nc.declare_dram_parameter("xkT", [D, S], BF16, isOutput=False)
    xvT = nc.declare_dram_parameter("xvT", [D, S], BF16, isOutput=False)
    mT = nc.declare_dram_parameter("mT", [S, S], BF16, isOutput=False)
    wq = nc.declare_dram_parameter("wq", [D, COLS], BF16, isOutput=False)
    wk = nc.declare_dram_parameter("wk", [D, COLS], BF16, isOutput=False)
    wv = nc.declare_dram_parameter("wv", [D, COLS], BF16, isOutput=False)
    wo = nc.declare_dram_parameter("wo", [COLS, D], BF16, isOutput=False)
    bq = nc.declare_dram_parameter("bq", [COLS, 1], F32, isOutput=False)
    bk = nc.declare_dram_parameter("bk", [COLS, 1], F32, isOutput=False)
    od = nc.declare_dram_parameter("od", [D, S], BF16, isOutput=True)

    xqT3 = xqT[:].rearrange("(n p) s -> n p s", p=P)
    xkT3 = xkT[:].rearrange("(n p) s -> n p s", p=P)
    xvT3 = xvT[:].rearrange("(n p) (h c) -> n p h c", p=P, c=SQB)
    mT3 = mT[:].rearrange("(n p) s -> n p s", p=P)
    # weights as single strided DMAs: partition = row-within-chunk, free =
    # (chunk, col) — one transfer per tensor instead of 8.
    wq3 = wq[:].rearrange("(n p) c -> p n c", p=P)
    wk3 = wk[:].rearrange("(n p) c -> p n c", p=P)
    wv3 = wv[:].rearrange("(n p) c -> p n c", p=P)
    wo3 = wo[:].rearrange("(n p) d -> p n d", p=P)
    bq3 = bq[:].rearrange("(n p) o -> p (n o)", p=P)
    bk3 = bk[:].rearrange("(n p) o -> p (n o)", p=P)
    od3 = od[:].rearrange("(n p) s -> n p s", p=P)

    with tile.TileContext(nc) as tc:
        _cms = []

        def open_pool(**kw):
            cm = tc.tile_pool(**kw)
            _cms.append(cm)
            return cm.__enter__()

        wpool = open_pool(name="wpool", bufs=1)
        xqkpool = open_pool(name="xqkpool", bufs=1)
        xvpool = open_pool(name="xvpool", bufs=1)
        qkpool = open_pool(name="qkpool", bufs=1)
        vpool = open_pool(name="vpool", bufs=1)
        maskpool = open_pool(name="maskpool", bufs=1)
        cpool = open_pool(name="cpool", bufs=1)
        pmpool = open_pool(name="pmpool", bufs=1)
        npool = open_pool(name="npool", bufs=1)
        opool = open_pool(name="opool", bufs=1)

        # ---- DMA plan ----
        # SP:   xq full chunks, late masks, xv halves, od out, transposes.
        # ACT:  xk full chunks + warm only; the exp stream owns it after.
        # Pool: consolidated weights/biases, early masks, evacuations.
        wq_all = wpool.tile([P, DK, COLS], BF16, name="wq_all")
        nc.gpsimd.dma_start(out=wq_all[:], in_=wq3)
        wk_all = wpool.tile([P, DK, COLS], BF16, name="wk_all")
        nc.gpsimd.dma_start(out=wk_all[:], in_=wk3)
        bqt = wpool.tile([P, COLS // P], F32, name="bqt")
        nc.gpsimd.dma_start(out=bqt[:], in_=bq3)
        bkt = wpool.tile([P, COLS // P], F32, name="bkt")
        nc.gpsimd.dma_start(out=bkt[:], in_=bk3)
        b_sb = {("bq", 0): bqt[:, 0:1], ("bq", 1): bqt[:, 1:2],
                ("bk", 0): bkt[:, 0:1], ("bk", 1): bkt[:, 1:2]}
        wq_sb = [wq_all[:, i, :] for i in range(DK)]
        wk_sb = [wk_all[:, i, :] for i in range(DK)]

        # xq as (dk, s-half) windows: the lead chains only need half 0
        # (2MB), so the first scores beat the serial-DMA window; half 1
        # (for the c1/sqb1 chains) is fetched behind the early masks.
        xq_sb, xk_sb = {}, []
        for i in range(DK):
            t = xqkpool.tile([P, SQB], BF16, name=f"xq{i}h0", tag="xq",
                             bufs=2 * DK)
            nc.sync.dma_start(out=t[:], in_=xqT3[i][:, 0:SQB])
            xq_sb[(i, 0)] = t
        for i in range(DK):
            t = xqkpool.tile([P, S], BF16, name=f"xk{i}", tag="xk", bufs=DK)
            nc.scalar.dma_start(out=t[:], in_=xkT3[i])
            xk_sb.append(t)

        def emit_xq_half1():
            for i in range(DK):
                t = xqkpool.tile([P, SQB], BF16, name=f"xq{i}h1", tag="xq",
                                 bufs=2 * DK)
                nc.sync.dma_start(out=t[:], in_=xqT3[i][:, SQB : 2 * SQB])
                xq_sb[(i, 1)] = t

        wv_all = wpool.tile([P, DK, COLS], BF16, name="wv_all")
        nc.gpsimd.dma_start(out=wv_all[:], in_=wv3)
        wv_sb = [wv_all[:, i, :] for i in range(DK)]
        ones = wpool.tile([P, 1], BF16, name="ones")
        nc.vector.memset(ones[:], 1.0)

        # warm the ACT exp table while ScalarE is still free
        warm = npool.tile([P, 1], F32, name="warm", tag="warm", bufs=1)
        nc.scalar.activation(
            warm[:], b_sb[("bq", 0)], mybir.ActivationFunctionType.Exp
        )

        # masks: split per sq-block so only block 0's half (4MB) competes
        # for DMA bandwidth in the critical early window; block 1's half is
        # fetched mid-stream.  All mask tiles share a 24-slot ring: block
        # 1's late tiles reuse block 0's early slots (their readers are
        # long done by the time the reusing DMA must land).  m[skc] must
        # land by first-exp + 1.07us*skc: skc 0/1 ride ACT right after
        # xk/warm, skc 2-5 ride Pool right behind the weights, the rest
        # alternate with the xv halves so xk keeps the DMA pipe early on.
        m_sb = {}
        M_BUFS = 24

        def mask_dma(sqb, i, eng):
            t = maskpool.tile([P, SQB], BF16, name=f"m{sqb}_sb{i}",
                              tag="m", bufs=M_BUFS)
            eng.dma_start(out=t[:], in_=mT3[i][:, sqb * SQB : (sqb + 1) * SQB])
            m_sb[(sqb, i)] = t

        for i in range(2):
            mask_dma(0, i, nc.gpsimd)

        # xv halves: octant o consumes half o//4 (bufs=8: each octant
        # needs all 8 dk tiles of its half at once).  hf0 on Pool paired
        # with masks m2-m5 so both land just in time; hf1 on SP behind
        # xq; m6-m15 follow on their queues at the consumption rate.
        xv_sb = {}
        for dk in range(DK):
            t = xvpool.tile([P, SQB], BF16, name=f"xv{dk}_0", tag="xv", bufs=8)
            nc.gpsimd.dma_start(out=t[:], in_=xvT3[dk, :, 0])
            xv_sb[(dk, 0)] = t
            if dk % 2 == 1 and dk < 8:
                mask_dma(0, 2 + dk // 2, nc.gpsimd)
        for dk in range(DK):
            t = xvpool.tile([P, SQB], BF16, name=f"xv{dk}_1", tag="xv", bufs=8)
            nc.sync.dma_start(out=t[:], in_=xvT3[dk, :, 1])
            xv_sb[(dk, 1)] = t
        for i in range(6, SKT):
            mask_dma(0, i, nc.sync)
        emit_xq_half1()
        wo_all = wpool.tile([P, COLS // P, D], BF16, name="wo_all")
        nc.gpsimd.dma_start(out=wo_all[:], in_=wo3)
        wo_sb = [wo_all[:, i, :] for i in range(COLS // P)]

        def emit_late_masks():
            for i in range(SKT):
                mask_dma(1, i, nc.sync if i % 2 else nc.gpsimd)

        concat_sb = [
            cpool.tile([P, S], BF16, name=f"concat_sb{i}")
            for i in range(COLS // P)
        ]
        qhT_sb = {
            (tn, c): qkpool.tile([P, S], BF16, name=f"{tn}hT{c}")
            for tn in ("q", "k") for c in range(COLS // P)
        }
        vh_sb = [
            vpool.tile([P, NH, HD], BF16, name=f"vh_sb{skt}")
            for skt in range(SKT)
        ]

        # ---------- psum pools ----------
        ps_s = open_pool(name="ps_s", bufs=1, space="PSUM")
        ps_pv = open_pool(name="ps_pv", bufs=1, space="PSUM")
        ps_dn = open_pool(name="ps_dn", bufs=1, space="PSUM")
        ps_pj_cm = tc.tile_pool(name="ps_pj", bufs=1, space="PSUM")
        _cms.append(ps_pj_cm)
        ps_pj = ps_pj_cm.__enter__()
        nonlocal_po = [None]  # opened after ps_pj closes
        _cms_po = []

        dn = ps_dn.tile([P, 2 * NH * SQC], F32, name="dn")

        # ---------- filler machinery ----------
        # fillers is a list of groups; each group is
        # {"units": [(cost, fn), ...], "earliest": chunk_index} and its
        # units are emitted in order, but a group is skipped until the
        # stream reaches `earliest` (so a unit whose input DMA lands late
        # never blocks the in-order PE queue and stalls the exp stream).
        fillers = []
        chain_done = set()    # (t, c, j) proj chains fully emitted
        vh_ready = [False] * SKT
        pv_backlog = []       # (sqb, h, skc, pv_tile, pm_tile)
        cur_chunk = [0]

        def add_group(units, earliest=0):
            fillers.append({"units": list(units), "earliest": earliest})

        def proj_chain(tn, c, j):
            """One [128c, 512s] projection chain + bias evac; returns units."""
            w_sb = wq_sb if tn == "q" else wk_sb
            state = {}

            def mm_unit(dk0, state=state, tn=tn, c=c, j=j):
                if "t" not in state:
                    state["t"] = ps_pj.tile([P, 512], F32, name="pj",
                                            tag="pj", bufs=2)
                for dk in (dk0, dk0 + 1):
                    if tn == "q":
                        rhs = xq_sb[(dk, j // 2)][
                            :, (j % 2) * 512 : (j % 2 + 1) * 512
                        ]
                    else:
                        rhs = xk_sb[dk][:, j * 512 : (j + 1) * 512]
                    nc.tensor.matmul(
                        state["t"][:],
                        lhsT=w_sb[dk][:, c * P : (c + 1) * P],
                        rhs=rhs,
                        start=(dk == 0),
                        stop=(dk == DK - 1),
                    )

            def evac_unit(state=state, tn=tn, c=c, j=j):
                nc.vector.tensor_scalar_add(
                    qhT_sb[(tn, c)][:, j * 512 : (j + 1) * 512],
                    state["t"][:], b_sb[(f"b{tn}", c)],
                )
                chain_done.add((tn, c, j))

            return [(430, lambda d=d: mm_unit(d)) for d in range(0, DK, 2)] + [
                (50, evac_unit)
            ]

        def octant(o):
            """V-proj for skt 2o, 2o+1: [128,512] bank, 2 chains."""
            state = {}

            def ms_unit(state=state):
                state["t"] = ps_pj.tile([P, 512], F32, name="pjv",
                                        tag="pj", bufs=2)
                nc.vector.memset(state["t"][:], 0.0)

            def mm_unit(dk0, state=state, o=o):
                for dk in (dk0, dk0 + 1):
                    for s in range(2):
                        skt = 2 * o + s
                        nc.tensor.matmul(
                            state["t"][:, s * COLS : (s + 1) * COLS],
                            lhsT=xv_sb[(dk, o // 4)][
                                :, (skt % 8) * P : (skt % 8 + 1) * P
                            ],
                            rhs=wv_sb[dk][:],
                            start=False,
                            stop=(dk == DK - 1),
                            skip_group_check=True,
                        )

            def evac_unit(s, state=state, o=o):
                skt = 2 * o + s
                nc.vector.tensor_copy(
                    vh_sb[skt][:],
                    state["t"][:, s * COLS : (s + 1) * COLS]
                    .rearrange("p (h d) -> p h d", h=NH),
                )
                vh_ready[skt] = True

            return ([(60, ms_unit)]
                    + [(220, lambda d=d: mm_unit(d)) for d in range(0, DK, 2)]
                    + [(80, lambda s=s: evac_unit(s)) for s in range(2)])

        def _pop_unit(eligible_only=True):
            for g in fillers:
                if eligible_only and g["earliest"] > cur_chunk[0]:
                    continue
                cost, fn = g["units"].pop(0)
                if not g["units"]:
                    fillers.remove(g)
                fn()
                return cost
            return None

        def drain(budget_ns):
            while fillers and budget_ns > 0:
                cost = _pop_unit()
                if cost is None:
                    return
                budget_ns -= cost

        def force_chain(tn, c, j):
            while (tn, c, j) not in chain_done:
                if _pop_unit(eligible_only=False) is None:
                    raise RuntimeError("force_chain ran out of fillers")

        def drain_pv_backlog():
            rest = []
            for item in pv_backlog:
                sqb, h, skc, pv_t, pm_t = item
                if pv_t is None:
                    pv_t = pv_tiles.get((sqb, h))
                if pv_t is None or not vh_ready[skc] or (sqb, h) not in memset_done:
                    rest.append(item)
                    continue
                dnc0 = sqb * NH * SQC + h * SQC
                for sqc in range(SQC):
                    nc.tensor.matmul(
                        pv_t[:, sqc * HD : (sqc + 1) * HD],
                        lhsT=pm_t[:, sqc * P : (sqc + 1) * P],
                        rhs=vh_sb[skc][:, h, :],
                        start=False,
                        stop=(skc == SKT - 1),
                        skip_group_check=True,
                    )
                    nc.tensor.matmul(
                        dn[:, dnc0 + sqc : dnc0 + sqc + 1],
                        lhsT=pm_t[:, sqc * P : (sqc + 1) * P],
                        rhs=ones[:],
                        start=False,
                        stop=(skc == SKT - 1),
                        skip_group_check=True,
                    )
            pv_backlog[:] = rest

        def force_pv(h_upto):
            """Emit all pending PV work for heads <= h_upto (forces octants)."""
            while any(not r for r in vh_ready):
                if _pop_unit(eligible_only=False) is None:
                    raise RuntimeError("force_pv ran out of fillers")
            drain_pv_backlog()
            assert not pv_backlog

        # ---------- lead-in: chains needed by the first chunks ----------
        lead = [("q", 0, 0), ("q", 0, 1), ("q", 1, 0), ("q", 1, 1),
                ("k", 0, 0), ("k", 0, 1), ("k", 0, 2), ("k", 0, 3)]
        for tn, c, j in lead:
            for cost, fn in proj_chain(tn, c, j):
                fn()

        # filler queue: V octants (gated on their xv half's estimated DMA
        # arrival so a pending transfer never blocks the PE queue), the k1
        # windows (xk has fully landed by the first drains), then sqb1's q
        # windows (gated on the xq half-1 fetch).
        for o in range(4):
            add_group(octant(o), earliest=0)
        for tn, c, j in [("k", 1, 0), ("k", 1, 1)]:
            add_group(proj_chain(tn, c, j))
        for o in range(4, 8):
            add_group(octant(o), earliest=XV1_CHUNK + 2 * (o - 4))
        for tn, c, j in [("k", 1, 2), ("k", 1, 3)]:
            add_group(proj_chain(tn, c, j))
        for tn, c, j in [("q", 0, 2), ("q", 0, 3), ("q", 1, 2), ("q", 1, 3)]:
            add_group(proj_chain(tn, c, j), earliest=XQ1_CHUNK)

        # ---------- chunk stream ----------
        chunks = [(sqb, h, skc)
                  for sqb in range(NSQB) for h in range(NH) for skc in range(SKT)]
        ss_tiles = {}
        pv_tiles = {}
        stg_live = [None]

        def emit_ss(sqb, h, skc):
            ht = h // 2
            hp = HD * (h % 2)
            q0 = sqb * SQB
            force_chain("q", ht, 2 * sqb)
            force_chain("q", ht, 2 * sqb + 1)
            force_chain("k", ht, skc // 4)
            ss = ps_s.tile([P, SQB], F32, name="pss", tag="pss", bufs=2)
            for i in range(SQB // 512):
                nc.tensor.matmul(
                    ss[:, i * 512 : (i + 1) * 512],
                    lhsT=qhT_sb[("k", ht)][hp : hp + HD, skc * P : (skc + 1) * P],
                    rhs=qhT_sb[("q", ht)][hp : hp + HD,
                                          q0 + i * 512 : q0 + (i + 1) * 512],
                    start=True,
                    stop=True,
                )
            ss_tiles[(sqb, h, skc)] = ss

        def emit_oproj(sqb):
            q0 = sqb * SQB
            last = sqb == NSQB - 1
            evac_engs = (
                [nc.vector, nc.vector, nc.vector, nc.scalar] if last
                else [nc.vector]
            )
            shared = {}
            ounits = []
            for dc in range(D // P):
                for sqh in range(SQB // 512):
                    u = dc * 2 + sqh
                    eng = evac_engs[u % len(evac_engs)]
                    dma_eng = nc.scalar if last and dc % 4 == 1 else nc.sync

                    def mm_unit(dc=dc, sqh=sqh, u=u, eng=eng, dma_eng=dma_eng):
                        if last and u % 4 >= 2:
                            # borrow the (now idle) score-psum banks for
                            # extra O-proj pipeline slots in the last block
                            if u % 4 == 2:
                                shared["t"] = ps_s.tile(
                                    [P, SQB], F32, name="pss", tag="pss",
                                    bufs=2
                                )
                            half = (u % 4) - 2
                            po = shared["t"][:, half * 512 : (half + 1) * 512]
                        else:
                            po = nonlocal_po[0].tile(
                                [P, 512], F32, name="po", tag="po", bufs=2
                            )
                        for cc in range(COLS // P):
                            nc.tensor.matmul(
                                po[:],
                                lhsT=wo_sb[cc][:, dc * P : (dc + 1) * P],
                                rhs=concat_sb[cc][
                                    :, q0 + sqh * 512 : q0 + (sqh + 1) * 512
                                ],
                                start=(cc == 0),
                                stop=(cc == COLS // P - 1),
                            )
                        if last:
                            # the pm ring is idle after the final mask-mul:
                            # borrow it as a deep evacuation ring, pairing
                            # the two halves of each od row block into one
                            # tile so there are 8 wide DMAs (split across
                            # SP and ACT) instead of 16 narrow ones
                            if sqh == 0:
                                shared["oev"] = pmpool.tile(
                                    [P, SQB], BF16, name="pm", tag="pm",
                                    bufs=8
                                )
                            oev = shared["oev"][:, sqh * 512 : (sqh + 1) * 512]
                            if eng is nc.scalar:
                                nc.scalar.copy(oev, po[:])
                            else:
                                eng.tensor_copy(oev, po[:])
                            if sqh == 1:
                                dma_eng.dma_start(
                                    out=od3[dc][:, q0 : q0 + SQB],
                                    in_=shared["oev"][:],
                                )
                        else:
                            oev = opool.tile([P, 512], BF16, name="oev",
                                             tag="oev", bufs=1)[:]
                            if eng is nc.scalar:
                                nc.scalar.copy(oev, po[:])
                            else:
                                eng.tensor_copy(oev, po[:])
                            dma_eng.dma_start(
                                out=od3[dc][
                                    :, q0 + sqh * 512 : q0 + (sqh + 1) * 512
                                ],
                                in_=oev,
                            )
                    ounits.append((520, mm_unit))
            add_group(ounits)

        norm_pending = []     # heads whose PV has finished streaming but
                              # whose normalization hasn't been emitted
        memset_done = set()   # (sqb, h) whose pv tile is zeroed
        state = {"po_opened": False}

        def start_head(sqb, h):
            if h == 0:
                nc.vector.memset(
                    dn[:, sqb * NH * SQC : (sqb + 1) * NH * SQC], 0.0
                )
            pv_t = ps_pv.tile([P, SQC * HD], F32, name="pv", tag="pv", bufs=1)
            nc.vector.memset(pv_t[:], 0.0)
            pv_tiles[(sqb, h)] = pv_t
            memset_done.add((sqb, h))

        def emit_norm(sqb, h, is_last):
            q0 = sqb * SQB
            ht = h // 2
            hp = HD * (h % 2)
            dnc0 = sqb * NH * SQC + h * SQC
            pv_t = pv_tiles.pop((sqb, h))
            rec = npool.tile([P, SQC], F32, name="rec", tag="rec", bufs=2)
            nc.vector.reciprocal_approx_fast(
                out=rec[:], in_=dn[:, dnc0 : dnc0 + SQC]
            )
            # heads are paired: the even head fills the lower 64 columns
            # of each 128-block of stg, the odd head the upper 64; one
            # full-width XBAR transpose per pair then writes the whole
            # concat^T chunk.  (128-partition outputs are the one form
            # CoreSim and the HW ucode agree on.)
            if h % 2 == 0:
                stg = npool.tile([P, SQB], BF16, name="stg", tag="stg",
                                 bufs=1)
                stg_live[0] = stg
            else:
                stg = stg_live[0]
            for sqc in range(SQC):
                eng = nc.scalar if is_last and sqc % 2 else nc.vector
                eng_ = eng
                if eng_ is nc.scalar:
                    nc.scalar.activation(
                        stg[:, sqc * P + hp : sqc * P + hp + HD],
                        pv_t[:, sqc * HD : (sqc + 1) * HD],
                        mybir.ActivationFunctionType.Copy,
                        scale=rec[:, sqc : sqc + 1],
                    )
                else:
                    nc.vector.tensor_scalar_mul(
                        stg[:, sqc * P + hp : sqc * P + hp + HD],
                        pv_t[:, sqc * HD : (sqc + 1) * HD],
                        rec[:, sqc : sqc + 1],
                    )
            if h % 2 == 1:
                if is_last:
                    # split across SP and ACT so the first half's O-proj
                    # can start while the second half still transposes
                    for tp, teng in ((0, nc.sync), (1, nc.scalar)):
                        teng.dma_start(
                            out=concat_sb[ht][
                                :, q0 + tp * 512 : q0 + (tp + 1) * 512
                            ].rearrange("p (j s) -> p j s", s=P),
                            in_=stg[:, tp * 512 : (tp + 1) * 512],
                            transpose=True,
                        )
                else:
                    nc.sync.dma_start(
                        out=concat_sb[ht][:, q0 : q0 + SQB]
                        .rearrange("p (j s) -> p j s", s=P),
                        in_=stg[:],
                        transpose=True,
                    )
            if h == NH - 1:
                if not state["po_opened"]:
                    # all proj/V fillers must be done before the proj
                    # psum pool can close and the O-proj pool open
                    while fillers:
                        _pop_unit(eligible_only=False)
                    ps_pj_cm.__exit__(None, None, None)
                    _cms.remove(ps_pj_cm)
                    ps_po_cm = tc.tile_pool(name="ps_po", bufs=1,
                                            space="PSUM")
                    _cms.append(ps_po_cm)
                    nonlocal_po[0] = ps_po_cm.__enter__()
                    _cms_po.append(ps_po_cm)
                    state["po_opened"] = True
                emit_oproj(sqb)

        def head_pv_clear(sqb, h):
            return (all(vh_ready)
                    and not any(it[0] == sqb and it[1] == h
                                for it in pv_backlog))

        def try_norms(force=False):
            while norm_pending:
                sqb, h = norm_pending[0]
                if force:
                    force_pv(h)
                if not head_pv_clear(sqb, h):
                    return
                norm_pending.pop(0)
                is_last = (sqb, h) == (NSQB - 1, NH - 1)
                emit_norm(sqb, h, is_last)
                # free slot: zero the next head's pv accumulator
                nxt = (sqb, h + 1) if h + 1 < NH else (sqb + 1, 0)
                if nxt[0] < NSQB:
                    start_head(*nxt)

        emit_ss(*chunks[0])
        start_head(0, 0)
        for ci, (sqb, h, skc) in enumerate(chunks):
            cur_chunk[0] = ci
            if ci + 1 < len(chunks):
                emit_ss(*chunks[ci + 1])
            if ci == 2 * SKT:
                emit_late_masks()
            if ci == SKT:
                emit_xq_half1()
            ss = ss_tiles.pop((sqb, h, skc))
            pm = pmpool.tile([P, SQB], BF16, name="pm", tag="pm", bufs=13)
            nc.scalar.activation(pm[:], ss[:], mybir.ActivationFunctionType.Exp)
            q0 = sqb * SQB
            # offload ~1/5 of the mask-muls to Pool (SBUF-only, legal on
            # gpsimd) so DVE stays under the ACT exp roofline
            meng = nc.gpsimd
            meng.tensor_mul(pm[:], pm[:], m_sb[(sqb, skc)][:])
            pv_backlog.append((sqb, h, skc, pv_tiles.get((sqb, h)), pm))
            drain(FILL_NS if ci < 2 * SKT else FILL_LATE_NS)
            drain_pv_backlog()
            if skc == SKT - 1:
                norm_pending.append((sqb, h))
            try_norms()
        try_norms(force=True)
        while fillers:
            _pop_unit(eligible_only=False)
        for cm in reversed(_cms):
            cm.__exit__(None, None, None)

    nc.compile()
    return nc


def get_program():
    if "nc" not in _CACHE:
        _CACHE["nc"] = _build_program()
    return _CACHE["nc"]


def make_in_maps(q, k, v, mask, Wq, bq, Wk, bk, Wv, bv, Wo, bo, gate):
    """Host-side sharding: per-core input dict (all numpy)."""
    q, k, v = (np.asarray(a, np.float32) for a in (q, k, v))
    mask = np.asarray(mask)
    Wq, bq, Wk, bk, Wv, bv, Wo, bo, gate = (
        np.asarray(a, np.float32) for a in (Wq, bq, Wk, bk, Wv, bv, Wo, bo, gate)
    )
    scale = 1.0 / np.sqrt(HD)
    xT = {}
    for b in range(B):
        xT[("q", b)] = np.ascontiguousarray(q[b].T).astype(NPBF16)
        xT[("k", b)] = np.ascontiguousarray(k[b].T).astype(NPBF16)
        xT[("v", b)] = np.ascontiguousarray(v[b].T).astype(NPBF16)
        xT[("m", b)] = np.ascontiguousarray(mask[b].T).astype(NPBF16)

    in_maps = []
    for c in range(NCORES):
        b = c // (NCORES // B)
        g = c % (NCORES // B)
        cols = slice(g * COLS, (g + 1) * COLS)
        gate_cols = np.repeat(gate[g * NH : (g + 1) * NH], HD)  # [256]
        in_maps.append(
            {
                "xqT": xT[("q", b)],
                "xkT": xT[("k", b)],
                "xvT": xT[("v", b)],
                "mT": xT[("m", b)],
                # fold the 1/sqrt(hd) score scale into Wq and bq;
                # fold the per-head gate into Wv (bv handled on host)
                "wq": (Wq[:, cols] * scale).astype(NPBF16),
                "wk": Wk[:, cols].astype(NPBF16),
                "wv": (Wv[:, cols] * gate_cols[None, :]).astype(NPBF16),
                "wo": np.ascontiguousarray(Wo[cols, :]).astype(NPBF16),
                "bq": (bq[cols] * scale).astype(np.float32).reshape(COLS, 1),
                "bk": bk[cols].astype(np.float32).reshape(COLS, 1),
            }
        )
    return in_maps


LAST_RESULTS = None


def kernel(q, k, v, mask, Wq, bq, Wk, bk, Wv, bv, Wo, bo, gate, trace=False):
    global LAST_RESULTS
    nc = get_program()
    in_maps = make_in_maps(q, k, v, mask, Wq, bq, Wk, bk, Wv, bv, Wo, bo, gate)
    res = run_bass_kernel_spmd(nc, in_maps, core_ids=list(range(NCORES)), trace=trace)
    LAST_RESULTS = res

    bv_ = np.asarray(bv, np.float32)
    bo_ = np.asarray(bo, np.float32)
    gate_ = np.asarray(gate, np.float32)
    Wo_ = np.asarray(Wo, np.float32)
    # attention rows sum to 1, so the bv term is a constant vector:
    # concat-level constant = repeat(gate, hd) * bv, projected through Wo.
    const = (np.repeat(gate_, HD) * bv_) @ Wo_ + bo_

    out = np.zeros((B, S, D), np.float32)
    for c in range(NCORES):
        b = c // (NCORES // B)
        out[b] += res.results[c]["od"].T.astype(np.float32)
    out += const[None, None, :]
    return out
